# revision 12
# baseline (speedup 1.0000x reference)
"""Local (windowed causal) attention Trainium2 kernel.

Problem: B=4, L=4096, D=1024, H=16 heads, dh=64, window W=128, causal
within each window. y = OutProj(Attn(QKV(x))).

Sharding: tokens are flattened to [16384, 1024] and split across 8
cores (2048 tokens = 16 complete windows per core). Fully data
parallel; weights are broadcast. No cross-core communication.

Per-core dataflow (activations kept transposed [feature, token] so the
contraction dim sits on SBUF partitions):
  A) x -> (PE transpose) -> xT;  qkvT[3072, 2048] = W3T-tiles (stationary,
     fp32r) x xT (moving, fp32r), bias+q-scale fused into the PSUM->SBUF
     copy on ACT, output cast to bf16.
  B) per (window w, head h):
       scores = Id.T@maskneg + qT.T@kT      (PSUM, fp32 accum)
       p = Exp(scores) on ACT, rowsums via accum_out
       pn = p * (1/rowsums)                 (DVE)
       pT = PE-transpose(pn); v = PE-transpose(vT-slice)
       aoT[64,128] = v.T @ pT               (bf16 matmul)
     then out-proj for the window: y[128,1024] = aoT.T @ WoT (fp32r),
     +bias via DVE broadcast add, DMA out.
"""

import numpy as np

import concourse.bass as bass
import concourse.mybir as mybir
import concourse.tile as tile
from concourse.bass_utils import run_bass_kernel_spmd
from concourse.vector_clock import ScopedClock, VectorClock

# ---------------------------------------------------------------------------
# Workaround: the pinned walrus rejects any sync-wait on an SP-engine CTRL
# (drain) instruction ("Too many sync wait commands"). Emit the end-of-kernel
# global-clock waits on non-SP engine drains instead, one wait per drain.
# ---------------------------------------------------------------------------


def _drain_and_barrier_split(self, tick_clock, wait_clock):
    g = tick_clock.global_clock
    engines = [self.nc.scalar, self.nc.vector, self.nc.gpsimd, self.nc.tensor]
    for p, t in enumerate(list(g)):
        if t == 0:
            continue
        part = VectorClock()
        part.require_at_least(p, t)
        d = engines[p % len(engines)].drain()
        wait_clock.add_sem_waits(d.ins, ScopedClock({None: part}))
    self.nc.sync.drain()
    self.nc.all_engine_barrier()
    assert self.sems is not None
    popped = self.nc._tile_sem_poison_stack.pop()
    assert popped is self._sem_poison
    self.nc.clear_and_free_semaphores(list(self.sems.allocated().values()))
    self.nc.all_engine_barrier()


tile.TileContext._drain_and_barrier = _drain_and_barrier_split


def _split_waits(nc, cap=1):
    """Hoist excess sync-waits onto standalone EventSemaphore instructions.

    The pinned walrus rejects instructions carrying more than one sync-wait
    command ("Too many sync wait commands"). Keep at most `cap` waits on each
    instruction and emit the rest as dedicated same-engine wait instructions
    immediately before it.
    """
    n = 0
    for f in nc.m.functions:
        for blk in f.blocks:
            out = []
            for inst in blk.instructions:
                si = inst.sync_info
                waits = list(si.on_wait) if si is not None and si.on_wait else []
                if len(waits) > cap:
                    keep = waits[-cap:] if cap else []
                    for wv in waits[: len(waits) - cap]:
                        n += 1
                        ev = mybir.InstEventSemaphore(
                            name=f"wsplit-{n}",
                            opcode="EventSemaphore",
                            engine=inst.engine,
                            debug=inst.debug,
                            ins=[],
                            outs=[],
                            descendants=None,
                            sync_info=mybir.SyncInfo(on_wait=[wv], on_update=[]),
                            bass_sim_breakpoint=False,
                            bass_priority=None,
                            bass_wait_until_ts=None,
                            bass_scheduled_tick=None,
                            bass_scheduled_proc=None,
                            bass_scheduled_scope=None,
                            bass_addl_debug=None,
                            bass_nofuse=True,
                        )
                        out.append(ev)
                    inst.sync_info = mybir.SyncInfo(
                        on_wait=keep, on_update=list(si.on_update)
                    )
                out.append(inst)
            blk.instructions[:] = out
    return n

# ---------------------------------------------------------------------------
# Shapes (hardcoded per spec)
# ---------------------------------------------------------------------------
B, L, D = 4, 4096, 1024
H, W = 16, 128
DH = D // H  # 64
N_CORES = 8
T = (B * L) // N_CORES  # 2048 tokens per core
NW = T // W  # 16 windows per core
KT = D // 128  # 8 k-tiles
NF3 = 3 * D // 128  # 24 feature tiles of qkv
TT = 4  # token chunks of 512 for the qkv matmul
TC = T // TT  # 512
SCALE = DH**-0.5  # 0.125

F32 = mybir.dt.float32
F32R = mybir.dt.float32r
BF16 = mybir.dt.bfloat16


def build_nc():
    nc = bass.Bass()

    x_in = nc.declare_dram_parameter("x", [T, D], F32, isOutput=False)
    # w3t_tiles[ft, kt, p, c] = qkv_w[ft*128 + c, kt*128 + p]
    w3t = nc.declare_dram_parameter("w3t", [NF3, KT, 128, 128], F32R, isOutput=False)
    # wot[kt, p, f] = out_w[f, kt*128 + p]
    wot_in = nc.declare_dram_parameter("wot", [KT, 128, D], F32R, isOutput=False)
    # b3[3072]; q part pre-scaled by SCALE on host
    b3_in = nc.declare_dram_parameter("b3", [3 * D], F32, isOutput=False)
    bo_in = nc.declare_dram_parameter("bo", [D], F32, isOutput=False)
    y_out = nc.declare_dram_parameter("y", [T, D], F32, isOutput=True)

    # Constants embedded in the NEFF
    ident_np = np.eye(128, dtype=np.float32)
    maskneg_np = np.triu(np.full((128, 128), -50.0, dtype=np.float32), k=1)
    ident_dram = nc.inline_tensor(ident_np, name="ident_c")
    maskneg_dram = nc.inline_tensor(maskneg_np, name="maskneg_c")

    with tile.TileContext(nc) as tc:
        with (
            tc.tile_pool(name="consts", bufs=1) as consts,
            tc.tile_pool(name="qkvt_res", bufs=1) as qkvt_pool,
        ):
            # --- constants ---
            id_f32 = consts.tile([128, 128], F32)
            nc.sync.dma_start(out=id_f32, in_=ident_dram[:])
            mask_f32 = consts.tile([128, 128], F32)
            nc.sync.dma_start(out=mask_f32, in_=maskneg_dram[:])
            id_bf16 = consts.tile([128, 128], BF16)
            nc.vector.tensor_copy(out=id_bf16, in_=id_f32)
            mask_bf16 = consts.tile([128, 128], BF16)
            nc.vector.tensor_copy(out=mask_bf16, in_=mask_f32)
            # b3 as [128, 24] (per-partition bias for qkvT feature tiles)
            b3_sb = consts.tile([128, NF3], F32)
            nc.gpsimd.dma_start(
                out=b3_sb, in_=b3_in[:].rearrange("(a p) -> p a", p=128)
            )
            # out bias broadcast to all partitions
            bo_sb = consts.tile([128, D], F32)
            bo_bcast = bass.AP(
                tensor=bo_in[:].tensor, offset=0, ap=[[0, 128], [1, D]]
            )
            nc.gpsimd.dma_start(out=bo_sb, in_=bo_bcast)

            # --- resident qkvT [3072 features, 2048 tokens] bf16 ---
            qkvt = [qkvt_pool.tile([128, T], BF16, name=f"qkvt{ft}") for ft in range(NF3)]

            # ============== Phase A: transpose x, QKV projection ==============
            with (
                tc.tile_pool(name="xt_res", bufs=1) as xt_pool,
                tc.tile_pool(name="xin", bufs=3) as xin_pool,
                tc.tile_pool(name="xp_ps", bufs=4, space="PSUM") as xp_ps,
                tc.tile_pool(name="w3sb", bufs=6) as w3sb_pool,
                tc.tile_pool(name="qkv_ps", bufs=3, space="PSUM") as qkv_ps,
            ):
                xt = [xt_pool.tile([128, T], F32R, name=f"xt{kt}") for kt in range(KT)]
                # load + transpose x
                for tt in range(T // 128):
                    x_sb = xin_pool.tile([128, D], F32)
                    nc.sync.dma_start(out=x_sb, in_=x_in[tt * 128 : (tt + 1) * 128, :])
                    for kt in range(KT):
                        xp = xp_ps.tile([128, 128], F32)
                        nc.tensor.transpose(
                            xp, x_sb[:, kt * 128 : (kt + 1) * 128], id_f32
                        )
                        nc.vector.tensor_copy(
                            out=xt[kt][:, tt * 128 : (tt + 1) * 128], in_=xp
                        )

                # QKV projection: qkvT[ft] = sum_kt w3t[ft,kt].T @ xT[kt]
                for ft in range(NF3):
                    w3_sb = w3sb_pool.tile([128, KT, 128], F32R, name="w3sb")
                    nc.sync.dma_start(
                        out=w3_sb, in_=w3t[ft].rearrange("k p c -> p k c")
                    )
                    for tt in range(TT):
                        ps = qkv_ps.tile([128, TC], F32)
                        for kt in range(KT):
                            nc.tensor.matmul(
                                ps,
                                w3_sb[:, kt, :],
                                xt[kt][:, tt * TC : (tt + 1) * TC],
                                start=(kt == 0),
                                stop=(kt == KT - 1),
                            )
                        # copy to SBUF bf16 with fused bias (+ q-scale, host-baked)
                        nc.scalar.activation(
                            out=qkvt[ft][:, tt * TC : (tt + 1) * TC],
                            in_=ps,
                            func=mybir.ActivationFunctionType.Identity,
                            bias=b3_sb[:, ft : ft + 1],
                            scale=SCALE if ft < KT else 1.0,
                        )

            # ============== Phase B: attention + output projection ==============
            with (
                tc.tile_pool(name="wot_res", bufs=1) as wot_pool,
                tc.tile_pool(name="sc_ps", bufs=2, space="PSUM") as sc_ps,
                tc.tile_pool(name="pt_ps", bufs=1, space="PSUM") as pt_ps,
                tc.tile_pool(name="vt_ps", bufs=1, space="PSUM") as vt_ps,
                tc.tile_pool(name="ao_ps", bufs=2, space="PSUM") as ao_ps,
                tc.tile_pool(name="y_ps", bufs=2, space="PSUM") as y_ps,
                tc.tile_pool(name="attn_sb", bufs=3) as attn_sb,
                tc.tile_pool(name="ao_sb", bufs=2) as ao_sb_pool,
                tc.tile_pool(name="y_sb", bufs=3) as y_sb_pool,
            ):
                wot_sb = wot_pool.tile([128, KT, D], F32R)
                nc.sync.dma_start(
                    out=wot_sb, in_=wot_in[:].rearrange("k p f -> p k f")
                )

                for w in range(NW):
                    c0, c1 = w * W, (w + 1) * W
                    aot = ao_sb_pool.tile([128, KT, W], F32R, name="aot")
                    for h in range(H):
                        ft, r0 = h // 2, (h % 2) * DH
                        qT = qkvt[ft][r0 : r0 + DH, c0:c1]
                        kT = qkvt[KT + ft][r0 : r0 + DH, c0:c1]
                        vT = qkvt[2 * KT + ft][r0 : r0 + DH, c0:c1]

                        # scores = mask + q.k^T  (PSUM fp32)
                        sc = sc_ps.tile([128, W], F32, name="sc")
                        nc.tensor.matmul(sc, id_bf16, mask_bf16, start=True, stop=False)
                        nc.tensor.matmul(sc, qT, kT, start=False, stop=True)

                        # p = exp(scores), rowsums fused
                        p_sb = attn_sb.tile([128, W], BF16, name="p_sb")
                        sums = attn_sb.tile([128, 1], F32, name="sums")
                        nc.scalar.activation(
                            out=p_sb,
                            in_=sc,
                            func=mybir.ActivationFunctionType.Exp,
                            accum_out=sums,
                        )
                        recip = attn_sb.tile([128, 1], F32, name="recip")
                        nc.vector.reciprocal(out=recip, in_=sums)
                        pn = attn_sb.tile([128, W], BF16, name="pn")
                        nc.vector.tensor_scalar_mul(out=pn, in0=p_sb, scalar1=recip)

                        # pT = pn.T (PE), v = vT.T (PE)
                        ptp = pt_ps.tile([128, W], BF16, name="ptp")
                        nc.tensor.transpose(ptp, pn, id_bf16)
                        pt_sb = attn_sb.tile([128, W], BF16, name="pt_sb")
                        nc.vector.tensor_copy(out=pt_sb, in_=ptp)

                        vp = vt_ps.tile([128, DH], BF16, name="vp")
                        nc.tensor.transpose(
                            vp, vT, id_bf16[r0 : r0 + DH, r0 : r0 + DH]
                        )
                        v_sb = attn_sb.tile([128, DH], BF16, name="v_sb")
                        nc.vector.tensor_copy(out=v_sb, in_=vp)

                        # aoT block [dh, qt] = v.T @ pT
                        ao = ao_ps.tile([DH, W], F32, name="ao")
                        nc.tensor.matmul(ao, v_sb, pt_sb, start=True, stop=True)
                        nc.vector.tensor_copy(
                            out=aot[r0 : r0 + DH, ft, :], in_=ao
                        )

                    # out-projection for this window
                    for fo in range(2):
                        f0, f1 = fo * 512, (fo + 1) * 512
                        yp = y_ps.tile([128, 512], F32, name="yp")
                        for kt in range(KT):
                            nc.tensor.matmul(
                                yp,
                                aot[:, kt, :],
                                wot_sb[:, kt, f0:f1],
                                start=(kt == 0),
                                stop=(kt == KT - 1),
                            )
                        ysb = y_sb_pool.tile([128, 512], F32, name="ysb")
                        nc.vector.tensor_add(out=ysb, in0=yp, in1=bo_sb[:, f0:f1])
                        nc.sync.dma_start(out=y_out[c0:c1, f0:f1], in_=ysb)

    _split_waits(nc)
    return nc


def prep_inputs(x, qkv_w, qkv_b, out_w, out_b):
    """Host-side prep: slice tokens per core, transpose/tile weights."""
    x = np.ascontiguousarray(np.asarray(x, dtype=np.float32).reshape(B * L, D))
    qkv_w = np.asarray(qkv_w, dtype=np.float32)
    qkv_b = np.asarray(qkv_b, dtype=np.float32)
    out_w = np.asarray(out_w, dtype=np.float32)
    out_b = np.asarray(out_b, dtype=np.float32)

    # w3t_tiles[ft, kt, p, c] = qkv_w[ft*128 + c, kt*128 + p]
    w3t = np.ascontiguousarray(
        qkv_w.reshape(NF3, 128, KT, 128).transpose(0, 2, 3, 1)
    )
    # wot[kt, p, f] = out_w[f, kt*128 + p]
    wot = np.ascontiguousarray(out_w.reshape(D, KT, 128).transpose(1, 2, 0))
    b3 = qkv_b.copy()
    b3[:D] *= SCALE

    in_maps = []
    for c in range(N_CORES):
        in_maps.append(
            {
                "x": x[c * T : (c + 1) * T],
                "w3t": w3t,
                "wot": wot,
                "b3": b3,
                "bo": out_b,
            }
        )
    return in_maps


_NC_CACHE = None


def kernel(x, qkv_w, qkv_b, out_w, out_b):
    global _NC_CACHE
    if _NC_CACHE is None:
        _NC_CACHE = build_nc()
    nc = _NC_CACHE
    in_maps = prep_inputs(x, qkv_w, qkv_b, out_w, out_b)
    res = run_bass_kernel_spmd(nc, in_maps, core_ids=list(range(N_CORES)))
    y = np.concatenate([res.results[c]["y"] for c in range(N_CORES)], axis=0)
    return y.reshape(B, L, D)


# revision 22
# speedup vs baseline: 19858.2402x; 19858.2402x over previous
"""Local (windowed causal) attention Trainium2 kernel.

Problem: B=4, L=4096, D=1024, H=16 heads, dh=64, window W=128, causal
within each window. y = OutProj(Attn(QKV(x))).

Sharding: tokens are flattened to [16384, 1024] and split across 8
cores (2048 tokens = 16 complete windows per core). Fully data
parallel; weights are broadcast. No cross-core communication.

Per-core dataflow (activations kept transposed [feature, token] so the
contraction dim sits on SBUF partitions):
  A) x -> (PE transpose) -> xT;  qkvT[3072, 2048] = W3T-tiles (stationary,
     fp32r) x xT (moving, fp32r), bias+q-scale fused into the PSUM->SBUF
     copy on ACT, output cast to bf16.
  B) per (window w, head h):
       scores = Id.T@maskneg + qT.T@kT      (PSUM, fp32 accum)
       p = Exp(scores) on ACT, rowsums via accum_out
       pn = p * (1/rowsums)                 (DVE)
       pT = PE-transpose(pn); v = PE-transpose(vT-slice)
       aoT[64,128] = v.T @ pT               (bf16 matmul)
     then out-proj for the window: y[128,1024] = aoT.T @ WoT (fp32r),
     +bias via DVE broadcast add, DMA out.
"""

import numpy as np

import concourse.bass as bass
import concourse.mybir as mybir
import concourse.tile as tile
from concourse.bass_utils import run_bass_kernel_spmd
from concourse.vector_clock import ScopedClock, VectorClock

# ---------------------------------------------------------------------------
# Workaround: the pinned walrus rejects any sync-wait on an SP-engine CTRL
# (drain) instruction ("Too many sync wait commands"). Emit the end-of-kernel
# global-clock waits on non-SP engine drains instead, one wait per drain.
# ---------------------------------------------------------------------------


def _drain_and_barrier_split(self, tick_clock, wait_clock):
    g = tick_clock.global_clock
    engines = [self.nc.scalar, self.nc.vector, self.nc.gpsimd, self.nc.tensor]
    for p, t in enumerate(list(g)):
        if t == 0:
            continue
        part = VectorClock()
        part.require_at_least(p, t)
        d = engines[p % len(engines)].drain()
        wait_clock.add_sem_waits(d.ins, ScopedClock({None: part}))
    self.nc.sync.drain()
    self.nc.all_engine_barrier()
    assert self.sems is not None
    popped = self.nc._tile_sem_poison_stack.pop()
    assert popped is self._sem_poison
    self.nc.clear_and_free_semaphores(list(self.sems.allocated().values()))
    self.nc.all_engine_barrier()


tile.TileContext._drain_and_barrier = _drain_and_barrier_split


def _split_waits(nc, cap=1):
    """Hoist excess sync-waits onto standalone EventSemaphore instructions.

    The pinned walrus rejects instructions carrying more than one sync-wait
    command ("Too many sync wait commands"). Keep at most `cap` waits on each
    instruction and emit the rest as dedicated same-engine wait instructions
    immediately before it.
    """
    n = 0
    for f in nc.m.functions:
        for blk in f.blocks:
            out = []
            for inst in blk.instructions:
                si = inst.sync_info
                waits = list(si.on_wait) if si is not None and si.on_wait else []
                if len(waits) > cap:
                    keep = waits[-cap:] if cap else []
                    for wv in waits[: len(waits) - cap]:
                        n += 1
                        ev = mybir.InstEventSemaphore(
                            name=f"wsplit-{n}",
                            opcode="EventSemaphore",
                            engine=inst.engine,
                            debug=inst.debug,
                            ins=[],
                            outs=[],
                            descendants=None,
                            sync_info=mybir.SyncInfo(on_wait=[wv], on_update=[]),
                            bass_sim_breakpoint=False,
                            bass_priority=None,
                            bass_wait_until_ts=None,
                            bass_scheduled_tick=None,
                            bass_scheduled_proc=None,
                            bass_scheduled_scope=None,
                            bass_addl_debug=None,
                            bass_nofuse=True,
                        )
                        out.append(ev)
                    inst.sync_info = mybir.SyncInfo(
                        on_wait=keep, on_update=list(si.on_update)
                    )
                out.append(inst)
            blk.instructions[:] = out
    return n

# ---------------------------------------------------------------------------
# Shapes (hardcoded per spec)
# ---------------------------------------------------------------------------
B, L, D = 4, 4096, 1024
H, W = 16, 128
DH = D // H  # 64
N_CORES = 8
T = (B * L) // N_CORES  # 2048 tokens per core
NW = T // W  # 16 windows per core
KT = D // 128  # 8 k-tiles
NF3 = 3 * D // 128  # 24 feature tiles of qkv
TT = 4  # token chunks of 512 for the qkv matmul
TC = T // TT  # 512
SCALE = DH**-0.5  # 0.125

F32 = mybir.dt.float32
F32R = mybir.dt.float32r
BF16 = mybir.dt.bfloat16


def build_nc(split_waits=True):
    nc = bass.Bass()

    x_in = nc.declare_dram_parameter("x", [T, D], F32, isOutput=False)
    # w3t_tiles[ft, kt, p, c] = qkv_w[ft*128 + c, kt*128 + p]
    w3t = nc.declare_dram_parameter("w3t", [NF3, KT, 128, 128], BF16, isOutput=False)
    # wot[kt, p, f] = out_w[f, kt*128 + p]
    wot_in = nc.declare_dram_parameter("wot", [KT, 128, D], F32R, isOutput=False)
    # b3[3072]; q part pre-scaled by SCALE on host
    b3_in = nc.declare_dram_parameter("b3", [3 * D], F32, isOutput=False)
    bo_in = nc.declare_dram_parameter("bo", [D], F32, isOutput=False)
    y_out = nc.declare_dram_parameter("y", [T, D], F32, isOutput=True)

    # Constants embedded in the NEFF
    ident_np = np.eye(128, dtype=np.float32)
    # 0/1 lower-triangular (incl diag) causal mask, repeated for 4 windows
    tril_np = np.tril(np.ones((128, 128), dtype=np.float32))
    mask4_np = np.ascontiguousarray(
        np.broadcast_to(tril_np[:, None, :], (128, 4, 128)).reshape(128, 512)
    )
    ident_dram = nc.inline_tensor(ident_np, name="ident_c")
    mask4_dram = nc.inline_tensor(mask4_np, name="mask4_c")

    with tile.TileContext(nc) as tc:
        with (
            tc.tile_pool(name="consts", bufs=1) as consts,
            tc.tile_pool(name="qkvt_res", bufs=1) as qkvt_pool,
        ):
            # --- constants ---
            id_f32 = consts.tile([128, 128], F32)
            nc.sync.dma_start(out=id_f32, in_=ident_dram[:])
            mask4_f32 = consts.tile([128, 512], F32)
            nc.sync.dma_start(out=mask4_f32, in_=mask4_dram[:])
            id_bf16 = consts.tile([128, 128], BF16)
            nc.vector.tensor_copy(out=id_bf16, in_=id_f32)
            mask4_bf16 = consts.tile([128, 512], BF16)
            nc.vector.tensor_copy(out=mask4_bf16, in_=mask4_f32)
            # b3 as [128, 24] (per-partition bias for qkvT feature tiles)
            b3_sb = consts.tile([128, NF3], F32)
            nc.gpsimd.dma_start(
                out=b3_sb, in_=b3_in[:].rearrange("(a p) -> p a", p=128)
            )
            # out bias broadcast to all partitions
            bo_sb = consts.tile([128, D], F32)
            bo_bcast = bass.AP(
                tensor=bo_in[:].tensor, offset=0, ap=[[0, 128], [1, D]]
            )
            nc.gpsimd.dma_start(out=bo_sb, in_=bo_bcast)

            # --- resident qkvT [3072 features, 2048 tokens] bf16 ---
            qkvt = [qkvt_pool.tile([128, T], BF16, name=f"qkvt{ft}") for ft in range(NF3)]

            # ============== Phase A: transpose x, QKV projection ==============
            with (
                tc.tile_pool(name="xt_res", bufs=1) as xt_pool,
                tc.tile_pool(name="xin", bufs=2) as xin_pool,
                tc.tile_pool(name="xp_ps", bufs=4, space="PSUM") as xp_ps,
                tc.tile_pool(name="w3sb", bufs=3) as w3sb_pool,
                tc.tile_pool(name="qkv_ps", bufs=3, space="PSUM") as qkv_ps,
            ):
                xt = [xt_pool.tile([128, T], BF16, name=f"xt{kt}") for kt in range(KT)]
                # load + transpose x; batch 4 transposed blocks per PSUM bank
                # so the PSUM->SBUF copy is one [128, 512] DVE op
                for t4 in range(T // 512):
                    x4 = xin_pool.tile([128, 4, D], F32, name="x4")
                    nc.sync.dma_start(
                        out=x4,
                        in_=x_in[t4 * 512 : (t4 + 1) * 512, :].rearrange(
                            "(i p) d -> p i d", p=128
                        ),
                    )
                    for kt in range(KT):
                        xp = xp_ps.tile([128, 512], F32)
                        for i in range(4):
                            nc.tensor.transpose(
                                xp[:, i * 128 : (i + 1) * 128],
                                x4[:, i, kt * 128 : (kt + 1) * 128],
                                id_f32,
                            )
                        nc.vector.tensor_copy(
                            out=xt[kt][:, t4 * 512 : (t4 + 1) * 512], in_=xp
                        )

                # QKV projection: qkvT[ft] = sum_kt w3t[ft,kt].T @ xT[kt]
                for ft in range(NF3):
                    w3_sb = w3sb_pool.tile([128, KT, 128], BF16, name="w3sb")
                    nc.sync.dma_start(
                        out=w3_sb, in_=w3t[ft].rearrange("k p c -> p k c")
                    )
                    for tt in range(TT):
                        ps = qkv_ps.tile([128, TC], F32)
                        for kt in range(KT):
                            nc.tensor.matmul(
                                ps,
                                w3_sb[:, kt, :],
                                xt[kt][:, tt * TC : (tt + 1) * TC],
                                start=(kt == 0),
                                stop=(kt == KT - 1),
                            )
                        # copy to SBUF bf16 with fused bias (+ q-scale, host-baked)
                        nc.scalar.activation(
                            out=qkvt[ft][:, tt * TC : (tt + 1) * TC],
                            in_=ps,
                            func=mybir.ActivationFunctionType.Identity,
                            bias=b3_sb[:, ft : ft + 1],
                            scale=SCALE if ft < KT else 1.0,
                        )

            # ============== Phase B: attention + output projection ==============
            # Processed per (head-pair hp, 4-window block wb):
            #   - scores for both heads of the pair via row-packed K=64
            #     matmuls (tile_position from base_partition 0/64), 4 windows
            #     side by side in one PSUM bank per sub-head
            #   - one exp per sub-head over [128, 512]
            #   - causal mask + normalization as multiplies on GPSIMD (idle)
            #   - P/V transposes batched 4-per-bank, single [128,512] copies
            #   - attn@V col-packed: both heads into one PSUM bank
            with (
                tc.tile_pool(name="wot_res", bufs=1) as wot_pool,
                tc.tile_pool(name="sc_ps", bufs=1, space="PSUM") as sc_ps,
                tc.tile_pool(name="pt_ps", bufs=2, space="PSUM") as pt_ps,
                tc.tile_pool(name="vt_ps", bufs=1, space="PSUM") as vt_ps,
                tc.tile_pool(name="ao_ps", bufs=1, space="PSUM") as ao_ps,
                tc.tile_pool(name="y_ps", bufs=2, space="PSUM") as y_ps,
                tc.tile_pool(name="attn_sb", bufs=2) as attn_sb,
                tc.tile_pool(name="ao_sb", bufs=2) as ao_sb_pool,
                tc.tile_pool(name="y_sb", bufs=3) as y_sb_pool,
            ):
                wot_sb = wot_pool.tile([128, KT, D], F32R)
                nc.sync.dma_start(
                    out=wot_sb, in_=wot_in[:].rearrange("k p f -> p k f")
                )

                HP = H // 2  # 8 head pairs == qkv k-tiles
                WB = NW // 4  # 4 window blocks

                for wb in range(WB):
                    b0 = wb * 4 * W  # token offset of this window block
                    aot4 = ao_sb_pool.tile([128, KT, 4 * W], F32R, name="aot4")
                    for hp in range(HP):
                        sc = [
                            sc_ps.tile([128, 512], F32, name=f"sc{s}") for s in range(2)
                        ]
                        for i in range(4):
                            c0 = b0 + i * W
                            for s in range(2):
                                r0 = s * DH
                                nc.tensor.matmul(
                                    sc[s][:, i * W : (i + 1) * W],
                                    qkvt[hp][r0 : r0 + DH, c0 : c0 + W],
                                    qkvt[KT + hp][r0 : r0 + DH, c0 : c0 + W],
                                    start=True,
                                    stop=True,
                                )
                        # p = exp(scores); mask + normalize on GPSIMD
                        p = [
                            attn_sb.tile([128, 512], BF16, name=f"p{s}")
                            for s in range(2)
                        ]
                        sums = attn_sb.tile([128, 2, 4], F32, name="sums")
                        recip = attn_sb.tile([128, 2, 4], F32, name="recip")
                        for s in range(2):
                            nc.scalar.activation(
                                out=p[s],
                                in_=sc[s],
                                func=mybir.ActivationFunctionType.Exp,
                            )
                            nc.gpsimd.tensor_mul(out=p[s], in0=p[s], in1=mask4_bf16)
                            nc.vector.reduce_sum(
                                out=sums[:, s, :],
                                in_=p[s].rearrange("p (i k) -> p i k", i=4),
                                axis=mybir.AxisListType.X,
                            )
                        nc.vector.reciprocal(
                            out=recip.rearrange("p a b -> p (a b)"),
                            in_=sums.rearrange("p a b -> p (a b)"),
                        )
                        for s in range(2):
                            for i in range(4):
                                nc.gpsimd.tensor_scalar_mul(
                                    out=p[s][:, i * W : (i + 1) * W],
                                    in0=p[s][:, i * W : (i + 1) * W],
                                    scalar1=recip[:, s, i : i + 1],
                                )

                        # pT = p.T per window (PE), batched into one bank/sub
                        pt_sb = []
                        for s in range(2):
                            ptp = pt_ps.tile([128, 512], BF16, name="ptp")
                            for i in range(4):
                                nc.tensor.transpose(
                                    ptp[:, i * W : (i + 1) * W],
                                    p[s][:, i * W : (i + 1) * W],
                                    id_bf16,
                                )
                            pts = attn_sb.tile([128, 512], BF16, name=f"pt{s}")
                            nc.vector.tensor_copy(out=pts, in_=ptp)
                            pt_sb.append(pts)

                        # v (both heads at once) = vT.T per window
                        vp = vt_ps.tile([128, 512], BF16, name="vp")
                        for i in range(4):
                            c0 = b0 + i * W
                            nc.tensor.transpose(
                                vp[:, i * W : (i + 1) * W],
                                qkvt[2 * KT + hp][:, c0 : c0 + W],
                                id_bf16,
                            )
                        v_sb = attn_sb.tile([128, 4, 2, DH], BF16, name="v_sb")
                        nc.vector.tensor_copy(
                            out=v_sb.rearrange("p a b c -> p (a b c)"), in_=vp
                        )

                        # aoT = v.T @ pT, both heads col-packed into one bank
                        ao = ao_ps.tile([128, 512], F32, name="ao")
                        for i in range(4):
                            for s in range(2):
                                nc.tensor.matmul(
                                    ao[s * DH : (s + 1) * DH, i * W : (i + 1) * W],
                                    v_sb[:, i, s, :],
                                    pt_sb[s][:, i * W : (i + 1) * W],
                                    start=True,
                                    stop=True,
                                    tile_position=(0, s * DH),
                                )
                        nc.vector.tensor_copy(out=aot4[:, hp, :], in_=ao)

                    # out-projection for the 4 windows of this block
                    for i in range(4):
                        c0 = b0 + i * W
                        for fo in range(2):
                            f0, f1 = fo * 512, (fo + 1) * 512
                            yp = y_ps.tile([128, 512], F32, name="yp")
                            for kt in range(KT):
                                nc.tensor.matmul(
                                    yp,
                                    aot4[:, kt, i * W : (i + 1) * W],
                                    wot_sb[:, kt, f0:f1],
                                    start=(kt == 0),
                                    stop=(kt == KT - 1),
                                )
                            ysb = y_sb_pool.tile([128, 512], F32, name="ysb")
                            nc.vector.tensor_add(
                                out=ysb, in0=yp, in1=bo_sb[:, f0:f1]
                            )
                            nc.sync.dma_start(
                                out=y_out[c0 : c0 + W, f0:f1], in_=ysb
                            )

    if split_waits:
        _split_waits(nc)
    return nc


def prep_inputs(x, qkv_w, qkv_b, out_w, out_b):
    """Host-side prep: slice tokens per core, transpose/tile weights."""
    x = np.ascontiguousarray(np.asarray(x, dtype=np.float32).reshape(B * L, D))
    qkv_w = np.asarray(qkv_w, dtype=np.float32)
    qkv_b = np.asarray(qkv_b, dtype=np.float32)
    out_w = np.asarray(out_w, dtype=np.float32)
    out_b = np.asarray(out_b, dtype=np.float32)

    import ml_dtypes

    # w3t_tiles[ft, kt, p, c] = qkv_w[ft*128 + c, kt*128 + p]
    w3t = np.ascontiguousarray(
        qkv_w.reshape(NF3, 128, KT, 128).transpose(0, 2, 3, 1)
    ).astype(ml_dtypes.bfloat16)
    # wot[kt, p, f] = out_w[f, kt*128 + p]
    wot = np.ascontiguousarray(out_w.reshape(D, KT, 128).transpose(1, 2, 0))
    b3 = qkv_b.copy()
    b3[:D] *= SCALE

    in_maps = []
    for c in range(N_CORES):
        in_maps.append(
            {
                "x": x[c * T : (c + 1) * T],
                "w3t": w3t,
                "wot": wot,
                "b3": b3,
                "bo": out_b,
            }
        )
    return in_maps


_NC_CACHE = None


def kernel(x, qkv_w, qkv_b, out_w, out_b):
    global _NC_CACHE
    if _NC_CACHE is None:
        _NC_CACHE = build_nc()
    nc = _NC_CACHE
    in_maps = prep_inputs(x, qkv_w, qkv_b, out_w, out_b)
    res = run_bass_kernel_spmd(nc, in_maps, core_ids=list(range(N_CORES)))
    y = np.concatenate([res.results[c]["y"] for c in range(N_CORES)], axis=0)
    return y.reshape(B, L, D)
